# revision 36
# baseline (speedup 1.0000x reference)
"""Trainium2 Bass kernel for nn_AttentionLayer (B=4, L=1024, C=768, H=12).

Sharding: 8 cores = 4 batches x 2 query-halves. Each core computes, for its
(batch b, query half):
  - q^T for its 512 queries, k^T/v for all 1024 keys (all 12 heads)
  - attention scores (softmax'd) for all heads, its query rows -> attn output
  - the output projection for its query rows -> out output
Zero cross-core communication; the host gathers per-core shards.

Layouts: matmuls contract over the partition dim, so the host passes x and the
weights pre-transposed (c-major) and pre-cast to bf16. Scores are computed
only in the transposed orientation (S^T = k q^T, m on partitions): that is
what the P@v matmul needs, and the attention-score output is written
transposed ([H, L, LQ] per core) with the host unshard permuting it back.
Row sums (softmax denominators) come from ones-matmuls accumulated alongside
the P@v matmuls; 1/r is broadcast across partitions on GpSimd and applied by
VectorE both to the score writes and to the attention output.

dtypes: bf16 matmul operands (fp32 PSUM accumulation), fp32 softmax
normalize and outputs. Head pairs share the PE array via row packing (K=64
S^T matmuls) and column packing (M=64 P@v matmuls, tile_position=(0,64)).
"""

import sys
import types

if "/opt/trn_rl_repo" not in sys.path:
    sys.path.insert(0, "/opt/trn_rl_repo")

import numpy as np

B, L, C, H, D = 4, 1024, 768, 12, 64
LQ = 512
NCORE = 8
SCALE = float(D) ** -0.5

_cache = {}


def _install_ntff_hook():
    """Make trace=True work under axon (exec_time_ns + perfetto)."""
    import concourse.bass_utils as bu

    try:
        from trn_agent_boot.trn_boot import _ntff_profile_via_ctypes

        hook = _ntff_profile_via_ctypes("/opt/axon/libaxon_pjrt.so")
    except Exception:
        hook = None
    m = types.ModuleType("antenv.axon_hooks")
    m.get_axon_ntff_profile_hook = lambda: hook
    m.set_axon_ntff_profile_hook = lambda h: None
    sys.modules["antenv.axon_hooks"] = m
    bu.upload_artifacts = lambda tmpdir: "local://" + tmpdir


def _build():
    import concourse.tile as tile
    from concourse import bacc, mybir

    f32 = mybir.dt.float32
    BF = mybir.dt.bfloat16
    Exp = mybir.ActivationFunctionType.Exp

    nc = bacc.Bacc("TRN2", target_bir_lowering=False, debug=False)
    xT_e = nc.dram_tensor("xT", [C, L], BF, kind="ExternalInput")
    xTq_e = nc.dram_tensor("xTq", [C, LQ], BF, kind="ExternalInput")
    wqkT_e = nc.dram_tensor("wqkT", [C, 2 * C], BF, kind="ExternalInput")
    wvT_e = nc.dram_tensor("wvT", [C, C], BF, kind="ExternalInput")
    wpT_e = nc.dram_tensor("wpT", [C, C], BF, kind="ExternalInput")
    bp_e = nc.dram_tensor("bp", [1, C], BF, kind="ExternalInput")
    cst_e = nc.dram_tensor("cst", [128, 128], BF, kind="ExternalInput")
    attn_e = nc.dram_tensor("attn_s", [H, L, LQ], f32, kind="ExternalOutput")
    out_e = nc.dram_tensor("out_s", [LQ, C], f32, kind="ExternalOutput")
    wsink = nc.dram_tensor("wsink", [128, 512], f32)

    xTq_r = xTq_e.ap().rearrange("(ko p) l -> p ko l", p=128)
    xT_r = xT_e.ap().rearrange("(ko p) l -> p ko l", p=128)
    wqkT_r = wqkT_e.ap().rearrange("(ko p) f -> p ko f", p=128)
    wvT_r = wvT_e.ap().rearrange("(ko p) f -> p ko f", p=128)

    with tile.TileContext(nc) as tc:
        with tc.tile_pool(name="persist", bufs=1) as persist:
            qT = persist.tile([128, 6, LQ], BF)  # chunk j: heads 2j,2j+1 (d on part)
            kT = persist.tile([128, 6, L], BF)
            v = persist.tile([128, 8, H, D], BF)  # [m-part, m-chunk, head, d]
            wp = persist.tile([128, 6, C], BF)
            bp = persist.tile([1, C], BF)
            ones = persist.tile([128, 128], BF)
            ao = persist.tile([128, 6, LQ], BF)  # attn-out^T chunks

            nc.sync.dma_start(out=ones[:], in_=cst_e.ap())
            nc.gpsimd.dma_start(
                out=wp[:], in_=wpT_e.ap().rearrange("(ko p) f -> p ko f", p=128)
            )
            nc.gpsimd.dma_start(out=bp[:], in_=bp_e.ap())

            # PE warm-up: dense matmuls during the input-load window pull the
            # HAM clock gate to 8/8 before the qkv burst begins.

            # ---------------- Phase 1: qkv projections ----------------
            with (
                tc.tile_pool(name="warm", bufs=1) as warm,
                tc.tile_pool(name="load", bufs=1) as ld,
                tc.tile_pool(name="pt", bufs=8) as ptp,
                tc.tile_pool(name="pT", bufs=5) as pTp,
                tc.tile_pool(name="sm", bufs=8) as sm,
                tc.tile_pool(name="nrm", bufs=2) as nrm,
                tc.tile_pool(name="outp", bufs=3) as outp,
                tc.tile_pool(name="psB", bufs=3, space="PSUM") as psB,
                tc.tile_pool(name="psSm", bufs=2, space="PSUM") as psSm,
            ):
                psS = psB
                ps1 = psSm
                psAV = psSm
                wps = psSm.tile([128, 512], f32, tag="s512")
                for wi in range(96):
                    nc.tensor.matmul(
                        wps[:, 0:128],
                        ones[:, 0:128],
                        ones[:, 0:128],
                        start=(wi == 0),
                        stop=(wi == 95),
                    )
                wsb = warm.tile([128, 128], f32, tag="wsb")
                nc.vector.tensor_copy(out=wsb[:], in_=wps[:, 0:128])
                nc.gpsimd.dma_start(out=wsink.ap()[:, 0:128], in_=wsb[:])

                xq_c, xk_c, wqk_c, wv_c = [], [], [], []
                for ck in range(6):
                    xq_t = ld.tile([128, LQ], BF, tag=f"xq{ck}")
                    xk_t = ld.tile([128, L], BF, tag=f"xk{ck}")
                    wqk_t = ld.tile([128, 2 * C], BF, tag=f"wqk{ck}")
                    wv_t = ld.tile([128, C], BF, tag=f"wv{ck}")
                    nc.sync.dma_start(out=xq_t[:], in_=xTq_r[:, ck, :])
                    nc.sync.dma_start(
                        out=wqk_t[:, 0:768], in_=wqkT_r[:, ck, 0:768]
                    )
                    xq_c.append(xq_t)
                    xk_c.append(xk_t)
                    wqk_c.append(wqk_t)
                    wv_c.append(wv_t)
                for ck in range(6):
                    nc.sync.dma_start(out=xk_c[ck][:], in_=xT_r[:, ck, :])
                    nc.sync.dma_start(
                        out=wqk_c[ck][:, 768:1536], in_=wqkT_r[:, ck, 768:1536]
                    )
                for ck in range(6):
                    nc.sync.dma_start(out=wv_c[ck][:], in_=wvT_r[:, ck, :])

                def emit_qk(j):
                    p = ps1.tile([128, 512], f32, tag="s512", name=f"q_{j}")
                    for ck in range(6):
                        nc.tensor.matmul(
                            p[:],
                            wqk_c[ck][:, j * 128 : (j + 1) * 128],
                            xq_c[ck][:],
                            start=(ck == 0),
                            stop=(ck == 5),
                        )
                    nc.scalar.copy(out=qT[:, j, :], in_=p[:])
                    for mb in range(2):
                        p = ps1.tile([128, 512], f32, tag="s512", name=f"k_{j}_{mb}")
                        for ck in range(6):
                            nc.tensor.matmul(
                                p[:],
                                wqk_c[ck][:, 768 + j * 128 : 768 + (j + 1) * 128],
                                xk_c[ck][:, mb * 512 : (mb + 1) * 512],
                                start=(ck == 0),
                                stop=(ck == 5),
                            )
                        nc.scalar.copy(
                            out=kT[:, j, mb * 512 : (mb + 1) * 512], in_=p[:]
                        )

                def emit_v(fb):
                    for mc in range(8):
                        p = ps1.tile([128, 512], f32, tag="s512", name=f"v_{fb}_{mc}")
                        for ck in range(6):
                            nc.tensor.matmul(
                                p[:, 0:384],
                                xk_c[ck][:, mc * 128 : (mc + 1) * 128],
                                wv_c[ck][:, fb * 384 : (fb + 1) * 384],
                                start=(ck == 0),
                                stop=(ck == 5),
                            )
                        nc.vector.tensor_copy(
                            out=v[:, mc, fb * 6 : (fb + 1) * 6, :].rearrange(
                                "p h d -> p (h d)"
                            ),
                            in_=p[:, 0:384],
                        )

                # ---------------- Phase 2: attention ----------------
                def emit_pair(hp):
                    # S^T = k q^T -> exp -> bf16, both heads interleaved so the
                    # K=64 matmuls of the pair row-pack concurrently in the array
                    pTs = [
                        pTp.tile([128, 8, 512], BF, tag="pT", name=f"pT_{hp}_0"),
                        pTp.tile([128, 8, 512], BF, tag="pT", name=f"pT_{hp}_1"),
                    ]
                    qsl = [qT[0:64, hp, :], qT[64:128, hp, :]]
                    ksl = [kT[0:64, hp, :], kT[64:128, hp, :]]
                    for mcp in range(4):
                        pss = [
                            psS.tile([128, 1024], f32, tag="psS", name=f"st_{hp}_{mcp}_0"),
                            psS.tile([128, 1024], f32, tag="psS", name=f"st_{hp}_{mcp}_1"),
                        ]
                        for jj in range(2):
                            mc = mcp * 2 + jj
                            for hh in range(2):
                                nc.tensor.matmul(
                                    pss[hh][:, jj * 512 : (jj + 1) * 512],
                                    ksl[hh][:, mc * 128 : (mc + 1) * 128],
                                    qsl[hh][:],
                                    start=True,
                                    stop=True,
                                )
                        for hh in range(2):
                            nc.scalar.activation(
                                out=pTs[hh][:, mcp * 2 : (mcp + 1) * 2, :],
                                in_=pss[hh][:],
                                func=Exp,
                            )
                    # av (col-tiled pair) + row sums rT via ones-matmuls
                    psav = psAV.tile([128, 512], f32, tag="s512")
                    psrt = psS.tile([128, 1024], f32, tag="psS")
                    for mc in range(8):
                        nc.tensor.matmul(
                            psav[0:64, :],
                            v[:, mc, 2 * hp, :],
                            pTs[0][:, mc, :],
                            start=(mc == 0),
                            stop=(mc == 7),
                        )
                        nc.tensor.matmul(
                            psav[64:128, :],
                            v[:, mc, 2 * hp + 1, :],
                            pTs[1][:, mc, :],
                            start=(mc == 0),
                            stop=(mc == 7),
                            tile_position=(0, 64),
                        )
                        nc.tensor.matmul(
                            psrt[0:1, 0:512],
                            ones[:, 0:1],
                            pTs[0][:, mc, :],
                            start=(mc == 0),
                            stop=(mc == 7),
                        )
                        nc.tensor.matmul(
                            psrt[0:1, 512:1024],
                            ones[:, 0:1],
                            pTs[1][:, mc, :],
                            start=(mc == 0),
                            stop=(mc == 7),
                        )
                    rt_sb = nrm.tile([1, 1024], f32, tag="rt_sb")
                    nc.scalar.copy(out=rt_sb[:], in_=psrt[0:1, :])
                    rbb = nrm.tile([128, 1024], f32, tag="rbb")
                    nc.gpsimd.partition_broadcast(out_ap=rbb[:], in_ap=rt_sb[:])
                    rib = nrm.tile([128, 1024], f32, tag="rib")
                    nc.vector.reciprocal_approx_fast(out=rib[:], in_=rbb[:])
                    nc.vector.tensor_mul(
                        ao[0:64, hp, :], psav[0:64, :], rib[0:64, 0:512]
                    )
                    nc.vector.tensor_mul(
                        ao[64:128, hp, :], psav[64:128, :], rib[64:128, 512:1024]
                    )
                    # normalized transposed score writes: P^T = pT * rib
                    for hh in range(2):
                        h = 2 * hp + hh
                        ribs = rib[:, hh * 512 : (hh + 1) * 512]
                        for mcp in range(4):
                            po = ptp.tile([128, 1024], f32, tag="po")
                            for jj in range(2):
                                nc.vector.tensor_tensor(
                                    po[:, jj * 512 : (jj + 1) * 512],
                                    pTs[hh][:, mcp * 2 + jj, :],
                                    ribs,
                                    mybir.AluOpType.mult,
                                )
                            weng = nc.sync if (mcp % 2 == 0) else nc.gpsimd
                            weng.dma_start(
                                out=attn_e.ap()[
                                    h, mcp * 256 : (mcp + 1) * 256, :
                                ].rearrange("(mc p) l -> p mc l", p=128),
                                in_=po[:].rearrange("p (mc l) -> p mc l", mc=2),
                            )

                # Staggered schedule: late qkv chunks fill PE idle slots in
                # the ACT-paced attention phase and keep the HAM clock warm.
                emit_qk(0)
                emit_qk(1)
                emit_qk(2)
                emit_v(0)
                emit_pair(0)
                emit_qk(3)
                emit_pair(1)
                emit_qk(4)
                emit_pair(2)
                emit_qk(5)
                emit_v(1)
                emit_pair(3)
                emit_pair(4)
                emit_pair(5)

                # ---------------- Phase 3: output projection ----------------
                for lqc in range(4):
                    for fb in range(2):
                        po = psS.tile([128, 1024], f32, tag="psS")
                        nc.tensor.matmul(
                            po[:, 0:384],
                            ones[0:1, :],
                            bp[0:1, fb * 384 : (fb + 1) * 384],
                            start=True,
                            stop=False,
                        )
                        for j in range(6):
                            nc.tensor.matmul(
                                po[:, 0:384],
                                ao[:, j, lqc * 128 : (lqc + 1) * 128],
                                wp[:, j, fb * 384 : (fb + 1) * 384],
                                start=False,
                                stop=(j == 5),
                            )
                        ot = outp.tile([128, 384], f32, tag="ot")
                        nc.scalar.copy(out=ot[:], in_=po[:, 0:384])
                        nc.sync.dma_start(
                            out=out_e.ap()[
                                lqc * 128 : (lqc + 1) * 128, fb * 384 : (fb + 1) * 384
                            ],
                            in_=ot[:],
                        )
    nc.compile()
    return nc


def _get_nc():
    if "nc" not in _cache:
        _install_ntff_hook()
        _cache["nc"] = _build()
    return _cache["nc"]


def _make_in_maps(x, w_qkv, w_proj, b_proj):
    import ml_dtypes

    bf16 = ml_dtypes.bfloat16
    wqkT_full = np.ascontiguousarray(w_qkv.T.astype(np.float32))  # [C, 3C]
    wqkT = wqkT_full[:, : 2 * C].copy()
    wqkT[:, :C] *= SCALE  # fold q scale into the weights
    wqkT = wqkT.astype(bf16)
    wvT = np.ascontiguousarray(wqkT_full[:, 2 * C :]).astype(bf16)
    wpT = np.ascontiguousarray(w_proj.T.astype(np.float32)).astype(bf16)
    bp = np.ascontiguousarray(b_proj.astype(np.float32)).reshape(1, C).astype(bf16)
    cst = np.ones((128, 128), bf16)
    in_maps = []
    for i in range(NCORE):
        b, half = i // 2, i % 2
        xT = np.ascontiguousarray(np.asarray(x[b]).T.astype(np.float32)).astype(bf16)
        xTq = np.ascontiguousarray(xT[:, half * LQ : (half + 1) * LQ])
        in_maps.append(
            {
                "xT": xT,
                "xTq": xTq,
                "wqkT": wqkT,
                "wvT": wvT,
                "wpT": wpT,
                "bp": bp,
                "cst": cst,
            }
        )
    return in_maps


def run(x, w_qkv, w_proj, b_proj, trace=False, tmpdir=None):
    from concourse.bass_utils import run_bass_kernel_spmd

    nc = _get_nc()
    in_maps = _make_in_maps(
        np.asarray(x), np.asarray(w_qkv), np.asarray(w_proj), np.asarray(b_proj)
    )
    res = run_bass_kernel_spmd(
        nc, in_maps, core_ids=list(range(NCORE)), trace=trace, tmpdir=tmpdir
    )
    out = np.empty((B, L, C), np.float32)
    attn = np.empty((B, H, L, L), np.float32)
    for i in range(NCORE):
        b, half = i // 2, i % 2
        attn[b, :, half * LQ : (half + 1) * LQ, :] = np.swapaxes(
            res.results[i]["attn_s"], 1, 2
        )
        out[b, half * LQ : (half + 1) * LQ, :] = res.results[i]["out_s"]
    return (out, attn), res


def kernel(x, w_qkv, w_proj, b_proj):
    (out, attn), _ = run(x, w_qkv, w_proj, b_proj, trace=False)
    return (out, attn)


# revision 37
# speedup vs baseline: 1.1489x; 1.1489x over previous
"""Trainium2 Bass kernel for nn_AttentionLayer (B=4, L=1024, C=768, H=12).

Sharding: 8 cores = 4 batches x 2 query-halves. Each core computes, for its
(batch b, query half):
  - q^T for its 512 queries, k^T/v for all 1024 keys (all 12 heads)
  - attention scores (softmax'd) for all heads, its query rows -> attn output
  - the output projection for its query rows -> out output
Zero cross-core communication; the host gathers per-core shards.

Layouts: matmuls contract over the partition dim, so the host passes x and the
weights pre-transposed (c-major) and pre-cast to bf16. Scores are computed
only in the transposed orientation (S^T = k q^T, m on partitions): that is
what the P@v matmul needs, and the attention-score output is written
transposed ([H, L, LQ] per core) with the host unshard permuting it back.
Row sums (softmax denominators) come from ones-matmuls accumulated alongside
the P@v matmuls; 1/r is broadcast across partitions on GpSimd and applied by
VectorE both to the score writes and to the attention output.

dtypes: bf16 matmul operands (fp32 PSUM accumulation), fp32 softmax
normalize and outputs. Head pairs share the PE array via row packing (K=64
S^T matmuls) and column packing (M=64 P@v matmuls, tile_position=(0,64)).
"""

import sys
import types

if "/opt/trn_rl_repo" not in sys.path:
    sys.path.insert(0, "/opt/trn_rl_repo")

import numpy as np

B, L, C, H, D = 4, 1024, 768, 12, 64
LQ = 512
NCORE = 8
SCALE = float(D) ** -0.5

_cache = {}


def _install_ntff_hook():
    """Make trace=True work under axon (exec_time_ns + perfetto)."""
    import concourse.bass_utils as bu

    try:
        from trn_agent_boot.trn_boot import _ntff_profile_via_ctypes

        hook = _ntff_profile_via_ctypes("/opt/axon/libaxon_pjrt.so")
    except Exception:
        hook = None
    m = types.ModuleType("antenv.axon_hooks")
    m.get_axon_ntff_profile_hook = lambda: hook
    m.set_axon_ntff_profile_hook = lambda h: None
    sys.modules["antenv.axon_hooks"] = m
    bu.upload_artifacts = lambda tmpdir: "local://" + tmpdir


def _build():
    import concourse.tile as tile
    from concourse import bacc, mybir

    f32 = mybir.dt.float32
    BF = mybir.dt.bfloat16
    Exp = mybir.ActivationFunctionType.Exp

    nc = bacc.Bacc("TRN2", target_bir_lowering=False, debug=False)
    xT_e = nc.dram_tensor("xT", [C, L], BF, kind="ExternalInput")
    xTq_e = nc.dram_tensor("xTq", [C, LQ], BF, kind="ExternalInput")
    wqkT_e = nc.dram_tensor("wqkT", [C, 2 * C], BF, kind="ExternalInput")
    wvT_e = nc.dram_tensor("wvT", [C, C], BF, kind="ExternalInput")
    wpT_e = nc.dram_tensor("wpT", [C, C], BF, kind="ExternalInput")
    bp_e = nc.dram_tensor("bp", [1, C], BF, kind="ExternalInput")
    cst_e = nc.dram_tensor("cst", [128, 128], BF, kind="ExternalInput")
    attn_e = nc.dram_tensor("attn_s", [H, L, LQ], f32, kind="ExternalOutput")
    out_e = nc.dram_tensor("out_s", [LQ, C], f32, kind="ExternalOutput")
    wsink = nc.dram_tensor("wsink", [128, 512], f32)

    xTq_r = xTq_e.ap().rearrange("(ko p) l -> p ko l", p=128)
    xT_r = xT_e.ap().rearrange("(ko p) l -> p ko l", p=128)
    wqkT_r = wqkT_e.ap().rearrange("(ko p) f -> p ko f", p=128)
    wvT_r = wvT_e.ap().rearrange("(ko p) f -> p ko f", p=128)

    with tile.TileContext(nc) as tc:
        with tc.tile_pool(name="persist", bufs=1) as persist:
            qT = persist.tile([128, 6, LQ], BF)  # chunk j: heads 2j,2j+1 (d on part)
            kT = persist.tile([128, 6, L], BF)
            v = persist.tile([128, 8, H, D], BF)  # [m-part, m-chunk, head, d]
            wp = persist.tile([128, 6, C], BF)
            bp = persist.tile([1, C], BF)
            ones = persist.tile([128, 128], BF)
            ao = persist.tile([128, 6, LQ], BF)  # attn-out^T chunks

            nc.sync.dma_start(out=ones[:], in_=cst_e.ap())
            nc.sync.dma_start(out=bp[:], in_=bp_e.ap())
            nc.sync.dma_start(
                out=wp[:], in_=wpT_e.ap().rearrange("(ko p) f -> p ko f", p=128)
            )

            # PE warm-up: dense matmuls during the input-load window pull the
            # HAM clock gate to 8/8 before the qkv burst begins.

            # ---------------- Phase 1: qkv projections ----------------
            with (
                tc.tile_pool(name="warm", bufs=1) as warm,
                tc.tile_pool(name="load", bufs=1) as ld,
                tc.tile_pool(name="pt", bufs=8) as ptp,
                tc.tile_pool(name="pT", bufs=5) as pTp,
                tc.tile_pool(name="sm", bufs=8) as sm,
                tc.tile_pool(name="nrm", bufs=2) as nrm,
                tc.tile_pool(name="outp", bufs=3) as outp,
                tc.tile_pool(name="psB", bufs=3, space="PSUM") as psB,
                tc.tile_pool(name="psSm", bufs=2, space="PSUM") as psSm,
            ):
                psS = psB
                ps1 = psSm
                psAV = psSm
                wps = psSm.tile([128, 512], f32, tag="s512")
                for wi in range(96):
                    nc.tensor.matmul(
                        wps[:, 0:128],
                        ones[:, 0:128],
                        ones[:, 0:128],
                        start=(wi == 0),
                        stop=(wi == 95),
                    )
                wsb = warm.tile([128, 128], f32, tag="wsb")
                nc.vector.tensor_copy(out=wsb[:], in_=wps[:, 0:128])
                nc.gpsimd.dma_start(out=wsink.ap()[:, 0:128], in_=wsb[:])

                xq_c, xk_c, wqk_c, wv_c = [], [], [], []
                for ck in range(6):
                    xq_t = ld.tile([128, LQ], BF, tag=f"xq{ck}")
                    xk_t = ld.tile([128, L], BF, tag=f"xk{ck}")
                    wqk_t = ld.tile([128, 2 * C], BF, tag=f"wqk{ck}")
                    wv_t = ld.tile([128, C], BF, tag=f"wv{ck}")
                    nc.sync.dma_start(out=xq_t[:], in_=xTq_r[:, ck, :])
                    nc.sync.dma_start(
                        out=wqk_t[:, 0:768], in_=wqkT_r[:, ck, 0:768]
                    )
                    xq_c.append(xq_t)
                    xk_c.append(xk_t)
                    wqk_c.append(wqk_t)
                    wv_c.append(wv_t)
                for ck in range(6):
                    nc.sync.dma_start(out=xk_c[ck][:], in_=xT_r[:, ck, :])
                    nc.sync.dma_start(
                        out=wqk_c[ck][:, 768:1536], in_=wqkT_r[:, ck, 768:1536]
                    )
                for ck in range(6):
                    nc.sync.dma_start(out=wv_c[ck][:], in_=wvT_r[:, ck, :])

                def emit_qk(j):
                    p = ps1.tile([128, 512], f32, tag="s512", name=f"q_{j}")
                    for ck in range(6):
                        nc.tensor.matmul(
                            p[:],
                            wqk_c[ck][:, j * 128 : (j + 1) * 128],
                            xq_c[ck][:],
                            start=(ck == 0),
                            stop=(ck == 5),
                        )
                    nc.scalar.copy(out=qT[:, j, :], in_=p[:])
                    for mb in range(2):
                        p = ps1.tile([128, 512], f32, tag="s512", name=f"k_{j}_{mb}")
                        for ck in range(6):
                            nc.tensor.matmul(
                                p[:],
                                wqk_c[ck][:, 768 + j * 128 : 768 + (j + 1) * 128],
                                xk_c[ck][:, mb * 512 : (mb + 1) * 512],
                                start=(ck == 0),
                                stop=(ck == 5),
                            )
                        nc.scalar.copy(
                            out=kT[:, j, mb * 512 : (mb + 1) * 512], in_=p[:]
                        )

                def emit_v(fb):
                    for mc in range(8):
                        p = ps1.tile([128, 512], f32, tag="s512", name=f"v_{fb}_{mc}")
                        for ck in range(6):
                            nc.tensor.matmul(
                                p[:, 0:384],
                                xk_c[ck][:, mc * 128 : (mc + 1) * 128],
                                wv_c[ck][:, fb * 384 : (fb + 1) * 384],
                                start=(ck == 0),
                                stop=(ck == 5),
                            )
                        nc.vector.tensor_copy(
                            out=v[:, mc, fb * 6 : (fb + 1) * 6, :].rearrange(
                                "p h d -> p (h d)"
                            ),
                            in_=p[:, 0:384],
                        )

                # ---------------- Phase 2: attention ----------------
                def emit_pair(hp):
                    # S^T = k q^T -> exp -> bf16, both heads interleaved so the
                    # K=64 matmuls of the pair row-pack concurrently in the array
                    pTs = [
                        pTp.tile([128, 8, 512], BF, tag="pT", name=f"pT_{hp}_0"),
                        pTp.tile([128, 8, 512], BF, tag="pT", name=f"pT_{hp}_1"),
                    ]
                    qsl = [qT[0:64, hp, :], qT[64:128, hp, :]]
                    ksl = [kT[0:64, hp, :], kT[64:128, hp, :]]
                    for mcp in range(4):
                        pss = [
                            psS.tile([128, 1024], f32, tag="psS", name=f"st_{hp}_{mcp}_0"),
                            psS.tile([128, 1024], f32, tag="psS", name=f"st_{hp}_{mcp}_1"),
                        ]
                        for jj in range(2):
                            mc = mcp * 2 + jj
                            for hh in range(2):
                                nc.tensor.matmul(
                                    pss[hh][:, jj * 512 : (jj + 1) * 512],
                                    ksl[hh][:, mc * 128 : (mc + 1) * 128],
                                    qsl[hh][:],
                                    start=True,
                                    stop=True,
                                )
                        for hh in range(2):
                            nc.scalar.activation(
                                out=pTs[hh][:, mcp * 2 : (mcp + 1) * 2, :],
                                in_=pss[hh][:],
                                func=Exp,
                            )
                    # av (col-tiled pair) + row sums rT via ones-matmuls
                    psav = psAV.tile([128, 512], f32, tag="s512")
                    psrt = psS.tile([128, 1024], f32, tag="psS")
                    for mc in range(8):
                        nc.tensor.matmul(
                            psav[0:64, :],
                            v[:, mc, 2 * hp, :],
                            pTs[0][:, mc, :],
                            start=(mc == 0),
                            stop=(mc == 7),
                        )
                        nc.tensor.matmul(
                            psav[64:128, :],
                            v[:, mc, 2 * hp + 1, :],
                            pTs[1][:, mc, :],
                            start=(mc == 0),
                            stop=(mc == 7),
                            tile_position=(0, 64),
                        )
                        nc.tensor.matmul(
                            psrt[0:1, 0:512],
                            ones[:, 0:1],
                            pTs[0][:, mc, :],
                            start=(mc == 0),
                            stop=(mc == 7),
                        )
                        nc.tensor.matmul(
                            psrt[0:1, 512:1024],
                            ones[:, 0:1],
                            pTs[1][:, mc, :],
                            start=(mc == 0),
                            stop=(mc == 7),
                        )
                    rt_sb = nrm.tile([1, 1024], f32, tag="rt_sb")
                    nc.scalar.copy(out=rt_sb[:], in_=psrt[0:1, :])
                    rbb = nrm.tile([128, 1024], f32, tag="rbb")
                    nc.gpsimd.partition_broadcast(out_ap=rbb[:], in_ap=rt_sb[:])
                    rib = nrm.tile([128, 1024], f32, tag="rib")
                    nc.vector.reciprocal_approx_fast(out=rib[:], in_=rbb[:])
                    nc.vector.tensor_mul(
                        ao[0:64, hp, :], psav[0:64, :], rib[0:64, 0:512]
                    )
                    nc.vector.tensor_mul(
                        ao[64:128, hp, :], psav[64:128, :], rib[64:128, 512:1024]
                    )
                    # normalized transposed score writes: P^T = pT * rib
                    for hh in range(2):
                        h = 2 * hp + hh
                        ribs = rib[:, hh * 512 : (hh + 1) * 512]
                        for mcp in range(4):
                            po = ptp.tile([128, 1024], f32, tag="po")
                            for jj in range(2):
                                nc.vector.tensor_tensor(
                                    po[:, jj * 512 : (jj + 1) * 512],
                                    pTs[hh][:, mcp * 2 + jj, :],
                                    ribs,
                                    mybir.AluOpType.mult,
                                )
                            weng = nc.sync if (mcp % 2 == 0) else nc.gpsimd
                            weng.dma_start(
                                out=attn_e.ap()[
                                    h, mcp * 256 : (mcp + 1) * 256, :
                                ].rearrange("(mc p) l -> p mc l", p=128),
                                in_=po[:].rearrange("p (mc l) -> p mc l", mc=2),
                            )

                # Staggered schedule: late qkv chunks fill PE idle slots in
                # the ACT-paced attention phase and keep the HAM clock warm.
                emit_qk(0)
                emit_qk(1)
                emit_qk(2)
                emit_v(0)
                emit_pair(0)
                emit_qk(3)
                emit_pair(1)
                emit_qk(4)
                emit_pair(2)
                emit_qk(5)
                emit_v(1)
                emit_pair(3)
                emit_pair(4)
                emit_pair(5)

                # ---------------- Phase 3: output projection ----------------
                for lqc in range(4):
                    for fb in range(2):
                        po = psS.tile([128, 1024], f32, tag="psS")
                        nc.tensor.matmul(
                            po[:, 0:384],
                            ones[0:1, :],
                            bp[0:1, fb * 384 : (fb + 1) * 384],
                            start=True,
                            stop=False,
                        )
                        for j in range(6):
                            nc.tensor.matmul(
                                po[:, 0:384],
                                ao[:, j, lqc * 128 : (lqc + 1) * 128],
                                wp[:, j, fb * 384 : (fb + 1) * 384],
                                start=False,
                                stop=(j == 5),
                            )
                        ot = outp.tile([128, 384], f32, tag="ot")
                        nc.scalar.copy(out=ot[:], in_=po[:, 0:384])
                        nc.sync.dma_start(
                            out=out_e.ap()[
                                lqc * 128 : (lqc + 1) * 128, fb * 384 : (fb + 1) * 384
                            ],
                            in_=ot[:],
                        )
    nc.compile()
    return nc


def _get_nc():
    if "nc" not in _cache:
        _install_ntff_hook()
        _cache["nc"] = _build()
    return _cache["nc"]


def _make_in_maps(x, w_qkv, w_proj, b_proj):
    import ml_dtypes

    bf16 = ml_dtypes.bfloat16
    wqkT_full = np.ascontiguousarray(w_qkv.T.astype(np.float32))  # [C, 3C]
    wqkT = wqkT_full[:, : 2 * C].copy()
    wqkT[:, :C] *= SCALE  # fold q scale into the weights
    wqkT = wqkT.astype(bf16)
    wvT = np.ascontiguousarray(wqkT_full[:, 2 * C :]).astype(bf16)
    wpT = np.ascontiguousarray(w_proj.T.astype(np.float32)).astype(bf16)
    bp = np.ascontiguousarray(b_proj.astype(np.float32)).reshape(1, C).astype(bf16)
    cst = np.ones((128, 128), bf16)
    in_maps = []
    for i in range(NCORE):
        b, half = i // 2, i % 2
        xT = np.ascontiguousarray(np.asarray(x[b]).T.astype(np.float32)).astype(bf16)
        xTq = np.ascontiguousarray(xT[:, half * LQ : (half + 1) * LQ])
        in_maps.append(
            {
                "xT": xT,
                "xTq": xTq,
                "wqkT": wqkT,
                "wvT": wvT,
                "wpT": wpT,
                "bp": bp,
                "cst": cst,
            }
        )
    return in_maps


def run(x, w_qkv, w_proj, b_proj, trace=False, tmpdir=None):
    from concourse.bass_utils import run_bass_kernel_spmd

    nc = _get_nc()
    in_maps = _make_in_maps(
        np.asarray(x), np.asarray(w_qkv), np.asarray(w_proj), np.asarray(b_proj)
    )
    res = run_bass_kernel_spmd(
        nc, in_maps, core_ids=list(range(NCORE)), trace=trace, tmpdir=tmpdir
    )
    out = np.empty((B, L, C), np.float32)
    attn = np.empty((B, H, L, L), np.float32)
    for i in range(NCORE):
        b, half = i // 2, i % 2
        attn[b, :, half * LQ : (half + 1) * LQ, :] = np.swapaxes(
            res.results[i]["attn_s"], 1, 2
        )
        out[b, half * LQ : (half + 1) * LQ, :] = res.results[i]["out_s"]
    return (out, attn), res


def kernel(x, w_qkv, w_proj, b_proj):
    (out, attn), _ = run(x, w_qkv, w_proj, b_proj, trace=False)
    return (out, attn)
